# revision 1
# baseline (speedup 1.0000x reference)
"""Causal self-attention (B=2, T=4096, D=512, H=8) on 8 Trainium2 NeuronCores.

Sharding: data parallel on batch (2 groups of 4 cores), tensor parallel on
heads (2 heads per core).  Each core:
  1. computes q/k/v for its 2 heads over the full T (using host-pretransposed
     x^T so the contraction dim lands on partitions),
  2. runs causal attention in a transposed layout: S^T[j,i] tiles from PE
     (bf16 operands), exp on ACT, row-sums via a ones-column appended to V,
  3. computes a partial output projection (its 128 rows of w_proj) per i-tile,
  4. two 4-core ReduceScatter(add) ops per batch group leave each core with
     the final y^T for t-tiles (rank) and (rank+4); the first RS overlaps the
     expensive late attention tiles.
Host reassembles the 8 cores x 2 [512, 512] y^T shards into [B, T, D].
"""

import os

import numpy as np

B, T, D = 2, 4096, 512
H = 8
DH = D // H  # 64
N_CORES = 8
TT = 512  # i-tile (query rows per tile)
JC = 128  # j-chunk (kv rows per chunk)
N_IT = T // TT  # 8
N_JC = T // JC  # 32
CC = 128  # contraction chunk
N_CC = D // CC  # 4

LAST_EXEC_NS = None
_CACHE = {}


def _build_program():
    from contextlib import ExitStack

    import concourse.mybir as mybir
    import concourse.tile as tile
    from concourse import bacc
    from concourse.masks import make_identity

    fp32 = mybir.dt.float32
    bf16 = mybir.dt.bfloat16
    Exp = mybir.ActivationFunctionType.Exp
    Log = mybir.ActivationFunctionType.Ln
    Copy = mybir.ActivationFunctionType.Copy

    nc = bacc.Bacc("TRN2", target_bir_lowering=False, debug=False,
                   num_devices=N_CORES)

    # ---- I/O -----------------------------------------------------------
    xT_d = nc.dram_tensor("xT", [D, T], bf16, kind="ExternalInput")
    wq_d = nc.dram_tensor("wq", [D, 128], bf16, kind="ExternalInput")
    wk_d = nc.dram_tensor("wk", [D, 128], bf16, kind="ExternalInput")
    wv_d = nc.dram_tensor("wv", [D, 128], bf16, kind="ExternalInput")
    bq_d = nc.dram_tensor("bq", [128, 1], fp32, kind="ExternalInput")
    bk_d = nc.dram_tensor("bk", [128, 1], fp32, kind="ExternalInput")
    bv_d = nc.dram_tensor("bv", [128, 1], fp32, kind="ExternalInput")
    msk_d = nc.dram_tensor("msk", [128, JC], bf16, kind="ExternalInput")
    blk2_d = nc.dram_tensor("blk2", [2, 128], bf16, kind="ExternalInput")
    wp_d = nc.dram_tensor("wp", [128, D], bf16, kind="ExternalInput")
    bp_d = nc.dram_tensor("bp", [128, N_CC], fp32, kind="ExternalInput")
    yT_d = nc.dram_tensor("yT", [D, 2 * TT], fp32, kind="ExternalOutput")

    with tile.TileContext(nc) as tc:
        with (
            tc.tile_pool(name="psum_mm", bufs=2, space="PSUM") as psum_mm,
            tc.tile_pool(name="psum_o", bufs=3, space="PSUM") as psum_o,
            tc.tile_pool(name="psum_bc", bufs=1, space="PSUM") as psum_bc,
            tc.tile_pool(name="ptiles", bufs=4) as ptiles,
            tc.tile_pool(name="small", bufs=4) as small,
            tc.tile_pool(name="ytiles", bufs=3) as ytiles,
            tc.tile_pool(name="dram", bufs=1, space="DRAM") as dram,
            ExitStack() as singles,
        ):
            def T_(shape, name, dt=bf16):
                t, free = tc.tile(shape, dt, name=name)
                singles.callback(free)
                return t

            # ---- persistent SBUF tensors -------------------------------
            xT_sb = T_([128, N_CC, T], "xT_sb")
            wq_sb = T_([128, N_CC, 128], "wq_sb")
            wk_sb = T_([128, N_CC, 128], "wk_sb")
            wv_sb = T_([128, N_CC, 128], "wv_sb")
            bq_sb = T_([128, 1], "bq_sb", fp32)
            bk_sb = T_([128, 1], "bk_sb", fp32)
            bv_sb = T_([128, 1], "bv_sb", fp32)
            msk_sb = T_([128, JC], "msk_sb")
            wp_sb = T_([128, D], "wp_sb")
            bp_sb = T_([128, N_CC], "bp_sb", fp32)
            qT_sb = T_([128, T], "qT_sb")
            kT_sb = T_([128, T], "kT_sb")
            # v^T first, then (after the transposes consume it) reused as
            # the attention output attn^T
            vT_sb = T_([128, T], "vT_sb")
            attnT_sb = vT_sb
            # V in natural layout [t-chunk, head, DH+1]; col 64 = ones
            V_sb = T_([128, N_JC, 2, DH + 1], "V_sb")
            ident = T_([128, 128], "ident")
            blk2 = T_([2, 128], "blk2")

            make_identity(nc, ident[:])
            nc.vector.memset(V_sb[:, :, :, DH], 1.0)

            # ---- load inputs -------------------------------------------
            for tt in range(N_IT):
                nc.sync.dma_start(
                    xT_sb[:, :, tt * TT:(tt + 1) * TT],
                    xT_d.ap()[:, tt * TT:(tt + 1) * TT]
                    .rearrange("(c p) t -> p c t", p=128),
                )
            for w_sb, w_d in ((wq_sb, wq_d), (wk_sb, wk_d), (wv_sb, wv_d)):
                nc.sync.dma_start(
                    w_sb[:], w_d.ap().rearrange("(c p) n -> p c n", p=128))
            for b_sb, b_d in ((bq_sb, bq_d), (bk_sb, bk_d), (bv_sb, bv_d)):
                nc.sync.dma_start(b_sb[:], b_d.ap())
            nc.sync.dma_start(msk_sb[:], msk_d.ap())
            nc.sync.dma_start(blk2[:], blk2_d.ap())
            nc.sync.dma_start(wp_sb[:], wp_d.ap())
            nc.sync.dma_start(bp_sb[:], bp_d.ap())

            # ---- QKV projections (q pre-scaled by 1/8 on host) ---------
            for tt in range(N_IT):
                sl = slice(tt * TT, (tt + 1) * TT)
                for w_sb, b_sb, dst in (
                    (wk_sb, bk_sb, kT_sb),
                    (wv_sb, bv_sb, vT_sb),
                    (wq_sb, bq_sb, qT_sb),
                ):
                    mm_ps = psum_mm.tile([128, TT], fp32, tag="mm")
                    for ci in range(N_CC):
                        nc.tensor.matmul(
                            mm_ps[:], w_sb[:, ci, :], xT_sb[:, ci, sl],
                            start=(ci == 0), stop=(ci == N_CC - 1))
                    nc.vector.tensor_scalar_add(dst[:, sl], mm_ps[:], b_sb[:])

            # ---- V: transpose v^T into natural [t, head, e] layout -----
            for jc in range(N_JC):
                tp_ps = psum_mm.tile([128, 128], bf16, tag="mm")
                nc.tensor.transpose(
                    tp_ps[:], vT_sb[:, jc * JC:(jc + 1) * JC], ident[:])
                for h in range(2):
                    nc.vector.tensor_copy(
                        V_sb[:, jc, h, 0:DH], tp_ps[:, h * DH:(h + 1) * DH])

            rs_in = [dram.tile([4, D, TT], bf16, name=f"rs_in{i}")
                     for i in range(2)]
            rs_out = [dram.tile([D, TT], bf16, name=f"rs_out{i}")
                      for i in range(2)]
            phi0_sb = T_([DH, 4, TT], "phi0_sb")
            phi1_sb = T_([DH, 4, TT], "phi1_sb")
            s_cat = T_([1, 8, TT], "s_cat")
            rec_cat = T_([1, 8, TT], "rec_cat")

            def emit_rs(half):
                nc.gpsimd.collective_compute(
                    "ReduceScatter", mybir.AluOpType.add,
                    replica_groups=[[0, 1, 2, 3], [4, 5, 6, 7]],
                    ins=[rs_in[half][:].opt()], outs=[rs_out[half][:].opt()])

            def emit_bias_out(half):
                for oc in range(N_CC):
                    yo_sb = ytiles.tile([128, TT], bf16, tag="yo")
                    nc.sync.dma_start(
                        yo_sb[:], rs_out[half][oc * 128:(oc + 1) * 128, :])
                    yb_sb = ytiles.tile([128, TT], fp32, tag="yb")
                    nc.vector.tensor_scalar_add(
                        yb_sb[:], yo_sb[:], bp_sb[:, oc:oc + 1])
                    nc.sync.dma_start(
                        yT_d.ap()[oc * 128:(oc + 1) * 128,
                                  half * TT:(half + 1) * TT],
                        yb_sb[:])

            # ---- attention, i-tile by i-tile; heads paired -------------
            for it in range(N_IT):
                isl = slice(it * TT, (it + 1) * TT)
                o_ps = [psum_o.tile([DH + 1, TT], fp32, tag="o",
                                    name=f"o_ps{h}") for h in range(2)]
                njc = 4 * (it + 1)
                for jc in range(njc):
                    d = jc - 4 * it  # >= 0 on diagonal chunks
                    lo = max(d, 0) * JC  # first valid i column
                    s_pair = psum_mm.tile([128, 2, TT], fp32, tag="mm")
                    for h in range(2):
                        hsl = slice(h * DH, (h + 1) * DH)
                        nc.tensor.matmul(
                            s_pair[:, h, lo:TT],
                            kT_sb[hsl, jc * JC:(jc + 1) * JC],
                            qT_sb[hsl, it * TT + lo:(it + 1) * TT],
                            start=True, stop=True, skip_group_check=True)
                    p_pair = ptiles.tile([128, 2, TT], bf16, tag="p")
                    nc.scalar.activation(p_pair[:, :, lo:TT],
                                         s_pair[:, :, lo:TT], Exp)
                    if d >= 0:  # diagonal chunk: causal mask
                        for h in range(2):
                            nc.vector.tensor_mul(
                                p_pair[:, h, lo:lo + JC],
                                p_pair[:, h, lo:lo + JC], msk_sb[:])
                    for h in range(2):
                        nc.tensor.matmul(
                            o_ps[h][:, lo:TT], V_sb[:, jc, h, :],
                            p_pair[:, h, lo:TT],
                            start=(jc == 0), stop=(jc == njc - 1),
                            skip_group_check=True)
                # stash unnormalized output + row-sums for batch normalize
                slot = it % 4
                for h, phi_h in ((0, phi0_sb), (1, phi1_sb)):
                    nc.vector.tensor_copy(
                        phi_h[:, slot, :], o_ps[h][0:DH, :])
                    nc.scalar.activation(
                        s_cat[0:1, slot * 2 + h, :],
                        o_ps[h][DH:DH + 1, :], Copy)

                if it % 4 == 3:
                    batch = it // 4
                    # one Ln + one Exp(-x) for 8 row-sum vectors: keeps the
                    # ACT table swaps down to 2 per batch
                    ln_t = small.tile([1, 8 * TT], fp32, tag="ln")
                    nc.scalar.activation(
                        ln_t[:], s_cat[0:1, :, :], Log)
                    nc.scalar.activation(
                        rec_cat[0:1, :, :], ln_t[:], Exp, scale=-1.0)
                    for itb in range(batch * 4, batch * 4 + 4):
                        ibsl = slice(itb * TT, (itb + 1) * TT)
                        for h, phi_h in ((0, phi0_sb), (1, phi1_sb)):
                            hsl = slice(h * DH, (h + 1) * DH)
                            bc_ps = psum_bc.tile([DH, TT], fp32, tag="bc")
                            nc.tensor.matmul(
                                bc_ps[:], blk2[0:1, 0:DH],
                                rec_cat[0:1, (itb % 4) * 2 + h, :],
                                start=True, stop=True)
                            bc_sb = small.tile([DH, TT], bf16, tag="bcs")
                            nc.vector.tensor_copy(bc_sb[:], bc_ps[:])
                            nc.vector.tensor_mul(
                                attnT_sb[hsl, ibsl],
                                phi_h[:, itb % 4, :], bc_sb[:])
                        # partial projection for this t-tile
                        for oc in range(N_CC):
                            y_ps = psum_mm.tile([128, TT], fp32, tag="mm")
                            nc.tensor.matmul(
                                y_ps[:], wp_sb[:, oc * 128:(oc + 1) * 128],
                                attnT_sb[:, ibsl], start=True, stop=True)
                            y_sb = ytiles.tile([128, TT], bf16, tag="y")
                            nc.vector.tensor_copy(y_sb[:], y_ps[:])
                            nc.sync.dma_start(
                                rs_in[batch][itb % 4,
                                             oc * 128:(oc + 1) * 128, :],
                                y_sb[:])
                    emit_rs(batch)
            emit_bias_out(0)
            emit_bias_out(1)

    nc.compile()
    return nc


def _prep_inputs(x, w_qkv, b_qkv, w_proj, b_proj):
    import ml_dtypes

    bf16 = ml_dtypes.bfloat16
    # [128, JC] lower-triangular-ish mask: mask[jrow, col] = 1 iff col >= jrow
    masks = (np.arange(JC)[None, :] >= np.arange(128)[:, None]).astype(bf16)
    in_maps = []
    for c in range(N_CORES):
        b, hp = divmod(c, 4)
        col = hp * 2 * DH  # first column of this core's 2 heads
        in_maps.append({
            "xT": np.ascontiguousarray(x[b].T).astype(bf16),
            "wq": (np.ascontiguousarray(w_qkv[:, col:col + 128])
                   * np.float32(0.125)).astype(bf16),
            "wk": np.ascontiguousarray(
                w_qkv[:, D + col:D + col + 128]).astype(bf16),
            "wv": np.ascontiguousarray(
                w_qkv[:, 2 * D + col:2 * D + col + 128]).astype(bf16),
            "bq": (b_qkv[col:col + 128] * np.float32(0.125)).reshape(128, 1).copy(),
            "bk": b_qkv[D + col:D + col + 128].reshape(128, 1).copy(),
            "bv": b_qkv[2 * D + col:2 * D + col + 128].reshape(128, 1).copy(),
            "msk": masks,
            "blk2": np.kron(np.eye(2), np.ones((1, DH))).astype(bf16),
            "wp": np.ascontiguousarray(w_proj[col:col + 128, :]).astype(bf16),
            "bp": np.ascontiguousarray(b_proj.reshape(N_CC, 128).T),
        })
    return in_maps


def kernel(x, w_qkv, b_qkv, w_proj, b_proj):
    global LAST_EXEC_NS
    from concourse.bass_utils import run_bass_kernel_spmd

    x = np.asarray(x, dtype=np.float32)
    w_qkv = np.asarray(w_qkv, dtype=np.float32)
    b_qkv = np.asarray(b_qkv, dtype=np.float32)
    w_proj = np.asarray(w_proj, dtype=np.float32)
    b_proj = np.asarray(b_proj, dtype=np.float32)

    if "nc" not in _CACHE:
        _CACHE["nc"] = _build_program()
    nc = _CACHE["nc"]

    in_maps = _prep_inputs(x, w_qkv, b_qkv, w_proj, b_proj)

    trace = bool(os.environ.get("BASS_KERNEL_TRACE"))
    kwargs = {}
    if trace:
        kwargs = {"trace": True,
                  "tmpdir": os.environ.get("BASS_KERNEL_TRACE_DIR") or None}
    res = run_bass_kernel_spmd(nc, in_maps, list(range(N_CORES)), **kwargs)
    LAST_EXEC_NS = res.exec_time_ns
    if trace:
        _CACHE["last_results"] = res

    # core c (group rank r = c%4) holds y^T for t-tiles r (cols 0:512) and
    # r+4 (cols 512:1024)
    out = np.empty((B, T, D), dtype=np.float32)
    for c in range(N_CORES):
        b, r = divmod(c, 4)
        yT = res.results[c]["yT"]
        out[b, r * TT:(r + 1) * TT, :] = yT[:, 0:TT].T
        out[b, (4 + r) * TT:(5 + r) * TT, :] = yT[:, TT:2 * TT].T
    return out

